# revision 3
# baseline (speedup 1.0000x reference)
"""Trainium2 Bass kernel for nn_ModelNew_3556232921999.

Pipeline: ConvTranspose3d(16->32, k=3, s=2, p=1, op=1) -> MaxPool3d(2)
          -> softmax(ch) -> subtract -> swish -> max(ch)

Key algebraic structure exploited:
  * convT(stride 2) + maxpool(2,2) => output spatial == input spatial, and the
    pool window {2m, 2m+1}^3 corresponds to the 8 parity classes of the convT.
    Each parity class is a small conv over x with taps at offsets {0,1}^3.
    pooled[c, m] = max over the 8 classes (+ bias, which commutes with max).
  * All 8 classes for one position come out of ONE matmul:
      lhsT = x-stack block [K=128 = (od,oh,ow,cin), M=128 positions]
      rhs  = W           [K=128, N=256 = (pd,ph,pw,c)]
      psum = [128 positions, 256]  -> channel dims land on the FREE axis,
    so the class-max, softmax and channel-max are all free-dim reductions.
  * swish/silu is quasiconvex => max_c silu(v_c) = max(silu(max_c v), silu(min_c v)).
    This removes the elementwise silu pass (only tiny [128, 2*16] silu per
    megagroup batch, all ACT work stays in the exp table-set family except two
    switches per batch).

Sharding: data-parallel over batch B=16 -> 2 per core x 8 cores.
"""

import os
import sys

sys.path.insert(0, "/opt/trn_rl_repo")

import numpy as np
import ml_dtypes

# ---------------------------------------------------------------- constants
IN_C, OUT_C, K, STRIDE, PAD, OUT_PAD = 16, 32, 3, 2, 1, 1
B, D, H, W = 16, 16, 64, 64
N_CORES = 8
B_PER_CORE = B // N_CORES  # 2

PLANE = H * W            # 4096 positions per (b, d) plane
BLK = 128                # positions per matmul block
BLKS_PER_PLANE = PLANE // BLK      # 32
GRP = 8                  # matmul blocks per psum group (1024 positions)
GRPS_PER_PLANE = BLKS_PER_PLANE // GRP  # 4
MG_GRPS = 2              # psum groups per megagroup (2048 positions)
MGS_PER_PLANE = GRPS_PER_PLANE // MG_GRPS  # 2

X_NP_DT = ml_dtypes.bfloat16   # x-stack storage dtype (DMA volume)
W_NP_DT = ml_dtypes.bfloat16   # conv weight dtype

_COMPILED = {}


def _tap(o, p):
    """Kernel tap index used by parity class p at window offset o, or None."""
    if p == 0:
        return 1 if o == 0 else None
    return 2 if o == 0 else 0


def build_wrhs(weight):
    """[128 rows=(od,oh,ow,cin), 256 cols=(pd,ph,pw,c)] conv matrix."""
    wr = np.zeros((2, 2, 2, IN_C, 2, 2, 2, OUT_C), dtype=np.float32)
    for od in range(2):
        for oh in range(2):
            for ow in range(2):
                for pd in range(2):
                    kd = _tap(od, pd)
                    if kd is None:
                        continue
                    for ph in range(2):
                        kh = _tap(oh, ph)
                        if kh is None:
                            continue
                        for pw in range(2):
                            kw = _tap(ow, pw)
                            if kw is None:
                                continue
                            # weight: [cin, cout, kd, kh, kw]
                            wr[od, oh, ow, :, pd, ph, pw, :] = weight[:, :, kd, kh, kw]
    return wr.reshape(128, 256)


def build_xstack(x):
    """[B, D, 128 rows=(od,oh,ow,cin), PLANE] shifted/padded copies of x."""
    xp = np.zeros((B, IN_C, D + 1, H + 1, W + 1), dtype=np.float32)
    xp[:, :, :D, :H, :W] = x
    S = np.empty((B, D, 2, 2, 2, IN_C, H, W), dtype=X_NP_DT)
    for od in range(2):
        for oh in range(2):
            for ow in range(2):
                # [B, cin, D, H, W] -> [B, D, cin, H, W]
                sl = xp[:, :, od:od + D, oh:oh + H, ow:ow + W]
                S[:, :, od, oh, ow] = sl.transpose(0, 2, 1, 3, 4).astype(X_NP_DT)
    return S.reshape(B, D, 128, PLANE)


def build_kernel():
    from concourse import bass, bacc, mybir, tile

    f32 = mybir.dt.float32
    bf16 = mybir.dt.bfloat16
    x_dt = bf16 if X_NP_DT == ml_dtypes.bfloat16 else f32
    w_dt = bf16 if W_NP_DT == ml_dtypes.bfloat16 else f32
    Alu = mybir.AluOpType
    Act = mybir.ActivationFunctionType
    Ax = mybir.AxisListType

    nc = bacc.Bacc("TRN2", target_bir_lowering=False, debug=False,
                   num_devices=N_CORES)

    xs_h = nc.declare_dram_parameter("xs", [B_PER_CORE, D, 128, PLANE], x_dt,
                                     isOutput=False)
    wr_h = nc.declare_dram_parameter("wr", [128, 256], w_dt, isOutput=False)
    bias_h = nc.declare_dram_parameter("biasrep", [128, 512], bf16,
                                       isOutput=False)
    sub_h = nc.declare_dram_parameter("subrep", [128, 512], f32,
                                      isOutput=False)
    y_h = nc.declare_dram_parameter("y", [B_PER_CORE, D, PLANE], f32,
                                    isOutput=True)

    with tile.TileContext(nc) as tc:
        with (
            tc.tile_pool(name="const", bufs=1) as constp,
            tc.tile_pool(name="xslab", bufs=3) as xpool,
            tc.tile_pool(name="psum", bufs=2, space="PSUM") as psump,
            tc.tile_pool(name="ev", bufs=2) as evp,
            tc.tile_pool(name="m1", bufs=2) as m1p,
            tc.tile_pool(name="m2", bufs=2) as m2p,
            tc.tile_pool(name="pooled", bufs=2) as plp,
            tc.tile_pool(name="pb", bufs=2) as pbp,
            tc.tile_pool(name="exp", bufs=2) as ep,
            tc.tile_pool(name="zr", bufs=3) as zp,
            tc.tile_pool(name="sm", bufs=2) as smp,
            tc.tile_pool(name="vv", bufs=2) as vp,
            tc.tile_pool(name="ext", bufs=2) as extp,
            tc.tile_pool(name="sil", bufs=2) as silp,
            tc.tile_pool(name="ost", bufs=2) as ostp,
        ):
            wr = constp.tile([128, 256], w_dt)
            nc.sync.dma_start(wr[:], wr_h[:, :])
            biasrep = constp.tile([128, 512], bf16)
            nc.sync.dma_start(biasrep[:], bias_h[:, :])
            subrep = constp.tile([128, 512], f32)
            nc.sync.dma_start(subrep[:], sub_h[:, :])

            for b in range(B_PER_CORE):
                # vmax/vmin staging for the whole batch-slice b:
                # [128, 2(ismin), D*MGS_PER_PLANE*16 = 512]
                ext = extp.tile([128, 2, D * MGS_PER_PLANE * 16], f32,
                                tag="ext")
                for d in range(D):
                    slab = xpool.tile([128, PLANE], x_dt, tag="slab")
                    nc.sync.dma_start(slab[:], xs_h[b, d])
                    for mg in range(MGS_PER_PLANE):
                        pooled = plp.tile([128, MG_GRPS, GRP, 32], bf16,
                                          tag="pooled")
                        for gl in range(MG_GRPS):
                            g = mg * MG_GRPS + gl
                            psum = psump.tile([128, GRP, 256], f32, tag="ps")
                            for k in range(GRP):
                                blk = (g * GRP + k) * BLK
                                nc.tensor.matmul(
                                    psum[:, k, :],
                                    slab[:, blk:blk + BLK],
                                    wr[:],
                                    start=True, stop=True,
                                )
                            # class-max tree (free dim): 8 -> 4 -> 2 -> 1
                            # (DVE may read only ONE operand from PSUM, so ACT
                            # first evacuates the pd=1 half to SBUF as bf16.)
                            ev = evp.tile([128, GRP, 128], bf16, tag="ev")
                            nc.scalar.activation(ev[:], psum[:, :, 128:256],
                                                 Act.Copy)
                            m1 = m1p.tile([128, GRP, 128], bf16, tag="m1")
                            nc.vector.tensor_tensor(
                                m1[:], psum[:, :, 0:128], ev[:], Alu.max)
                            m2 = m2p.tile([128, GRP, 64], bf16, tag="m2")
                            nc.vector.tensor_tensor(
                                m2[:], m1[:, :, 0:64], m1[:, :, 64:128],
                                Alu.max)
                            nc.vector.tensor_tensor(
                                pooled[:, gl], m2[:, :, 0:32], m2[:, :, 32:64],
                                Alu.max)
                        # ---- megagroup stages on [128, 512] ----
                        pb = pbp.tile([128, 512], bf16, tag="pb")
                        nc.vector.tensor_tensor(
                            pb[:],
                            pooled[:].rearrange("p a b c -> p (a b c)"),
                            biasrep[:], Alu.add)
                        E = ep.tile([128, 16, 32], f32, tag="E")
                        nc.scalar.activation(
                            E[:].rearrange("p a b -> p (a b)"), pb[:], Act.Exp)
                        Z = zp.tile([128, 16], f32, tag="Z")
                        nc.vector.tensor_reduce(Z[:], E[:], axis=Ax.X,
                                                op=Alu.add)
                        R = zp.tile([128, 16], f32, tag="R")
                        nc.vector.reciprocal(R[:], Z[:])
                        sm = smp.tile([128, 16, 32], f32, tag="sm")
                        nc.gpsimd.tensor_tensor(
                            sm[:], E[:],
                            R[:].unsqueeze(2).broadcast_to([128, 16, 32]),
                            Alu.mult)
                        v = vp.tile([128, 16, 32], f32, tag="v")
                        nc.gpsimd.tensor_tensor(
                            v[:],
                            sm[:],
                            subrep[:].rearrange("p (a b) -> p a b", a=16, b=32),
                            Alu.subtract)
                        col = (d * MGS_PER_PLANE + mg) * 16
                        nc.vector.tensor_reduce(
                            ext[:, 0, col:col + 16], v[:], axis=Ax.X,
                            op=Alu.max)
                        nc.vector.tensor_reduce(
                            ext[:, 1, col:col + 16], v[:], axis=Ax.X,
                            op=Alu.min)
                # ---- per-b tail: tiny silu + final pairwise max ----
                sil = silp.tile([128, 2, 512], f32, tag="sil")
                nc.scalar.activation(
                    sil[:].rearrange("p a b -> p (a b)"),
                    ext[:].rearrange("p a b -> p (a b)"), Act.Silu)
                ost = ostp.tile([128, 512], f32, tag="ost")
                nc.vector.tensor_tensor(ost[:], sil[:, 0, :], sil[:, 1, :],
                                        Alu.max)
                # ost[p, (d, mg, j)] ; j=(gl, blk) ; plane pos = (mg*2+gl)*1024
                #   + blk*128 + p
                nc.sync.dma_start(
                    y_h[b].flatten().rearrange(
                        "(dd hg blk p) -> p dd hg blk",
                        dd=D, hg=4, blk=GRP, p=BLK),
                    ost[:].rearrange("p (dd hg blk) -> p dd hg blk",
                                     dd=D, hg=4, blk=GRP))

    nc.compile()
    return nc


def _get_nc():
    if "nc" not in _COMPILED:
        _COMPILED["nc"] = build_kernel()
    return _COMPILED["nc"]


LAST_EXEC_NS = None


def build_in_maps(xs, wr, bias, subtract):
    # biasrep: [128, 512] pattern (16 blocks x 32 ch), bf16
    biasrep = np.tile(bias[None, None, :], (128, 16, 1)).reshape(128, 512)
    biasrep = biasrep.astype(ml_dtypes.bfloat16)
    subrep = np.tile(subtract[None, None, :], (128, 16, 1)).reshape(
        128, 512).astype(np.float32)

    in_maps = []
    for c in range(N_CORES):
        in_maps.append({
            "xs": np.ascontiguousarray(xs[c * B_PER_CORE:(c + 1) * B_PER_CORE]),
            "wr": wr,
            "biasrep": biasrep,
            "subrep": subrep,
        })
    return in_maps


def kernel(x, weight, bias, subtract):
    from concourse.bass_utils import run_bass_kernel_spmd

    x = np.asarray(x, dtype=np.float32)
    weight = np.asarray(weight, dtype=np.float32)
    bias = np.asarray(bias, dtype=np.float32)
    subtract = np.asarray(subtract, dtype=np.float32)

    nc = _get_nc()

    xs = build_xstack(x)                      # [B, D, 128, PLANE]
    wr = build_wrhs(weight).astype(W_NP_DT)   # [128, 256]
    in_maps = build_in_maps(xs, wr, bias, subtract)

    res = run_bass_kernel_spmd(nc, in_maps, core_ids=list(range(N_CORES)))
    outs = [res.results[c]["y"].reshape(B_PER_CORE, D, H, W)
            for c in range(N_CORES)]
    return np.concatenate(outs, axis=0)



# revision 12
# speedup vs baseline: 3.1632x; 3.1632x over previous
"""Trainium2 Bass kernel for nn_ModelNew_3556232921999.

Pipeline: ConvTranspose3d(16->32, k=3, s=2, p=1, op=1) -> MaxPool3d(2)
          -> softmax(ch) -> subtract -> swish -> max(ch)

Key algebraic structure exploited:
  * convT(stride 2) + maxpool(2,2) => output spatial == input spatial, and the
    pool window {2m, 2m+1}^3 corresponds to the 8 parity classes of the convT.
    Each parity class is a small conv over x with taps at offsets {0,1}^3.
    pooled[c, m] = max over the 8 classes (+ bias, which commutes with max).
  * All parity classes for one position come out of matmuls with
      lhsT = x-stack block [K=128 = (od,oh,ow,cin), M=128 positions]
    so channel/class dims land on the FREE axis and the softmax reductions
    are free-dim reductions.
  * Pair-max via ReLU identity: max(a,b) = b + relu(a-b). The pd-pair of
    classes is computed as D = x@(W0-W1) (one matmul), relu'd IN PLACE in
    PSUM by the scalar engine (PE's has_written bits survive engine writes),
    then B = x@W1 is accumulated on top with start=False. One segmented
    tensor_reduce(max) over the remaining 4 (ph,pw) candidates then yields
    the pooled value -- no DVE max tree.
  * swish/silu is quasiconvex => max_c silu(v_c) = max(silu(max_c v),
    silu(min_c v)), so silu runs on 2 values per position, not 32.

Sharding: data-parallel over batch B=16 -> 2 per core x 8 cores.
"""

import os
import sys

sys.path.insert(0, "/opt/trn_rl_repo")

import numpy as np
import ml_dtypes

# ---------------------------------------------------------------- constants
IN_C, OUT_C, K, STRIDE, PAD, OUT_PAD = 16, 32, 3, 2, 1, 1
B, D, H, W = 16, 16, 64, 64
N_CORES = 8
B_PER_CORE = B // N_CORES  # 2

PLANE = H * W            # 4096 positions per (b, d) plane
BLK = 128                # positions per matmul block
BLKS_PER_PLANE = PLANE // BLK      # 32
GRP = 8                  # matmul blocks per psum group (2-bank tile)
GRPS_PER_PLANE = BLKS_PER_PLANE // GRP  # 2

X_NP_DT = ml_dtypes.bfloat16   # x-stack storage dtype (DMA volume)
W_NP_DT = ml_dtypes.bfloat16   # conv weight dtype

_COMPILED = {}


def _tap(o, p):
    """Kernel tap index used by parity class p at window offset o, or None."""
    if p == 0:
        return 1 if o == 0 else None
    return 2 if o == 0 else 0


def build_w8(weight):
    """[128 rows=(od,oh,ow,cin), 2,2,2,32 cols=(pd,ph,pw,c)] conv matrix."""
    wr = np.zeros((2, 2, 2, IN_C, 2, 2, 2, OUT_C), dtype=np.float32)
    for od in range(2):
        for oh in range(2):
            for ow in range(2):
                for pd in range(2):
                    kd = _tap(od, pd)
                    if kd is None:
                        continue
                    for ph in range(2):
                        kh = _tap(oh, ph)
                        if kh is None:
                            continue
                        for pw in range(2):
                            kw = _tap(ow, pw)
                            if kw is None:
                                continue
                            # weight: [cin, cout, kd, kh, kw]
                            wr[od, oh, ow, :, pd, ph, pw, :] = weight[:, :, kd, kh, kw]
    return wr.reshape(128, 2, 2, 2, OUT_C)


def build_wrhs(weight):
    """[128, 256] = [D-half | B-half], col within half = (c, j=(ph,pw)).

    D = W_pd0 - W_pd1 (pair differences), B = W_pd1 (pair base), so that
    pairmax = B + relu(D)."""
    w8 = build_w8(weight)                      # [128, pd, ph, pw, c]
    # [128, ph, pw, c] -> order cols (c, j): transpose to [128, c, ph, pw]
    w0 = w8[:, 0].transpose(0, 3, 1, 2).reshape(128, 128)
    w1 = w8[:, 1].transpose(0, 3, 1, 2).reshape(128, 128)
    return np.concatenate([w0 - w1, w1], axis=1)  # [128, 256]


def build_xstack(x):
    """[B, D, 128 rows=(od,oh,ow,cin), PLANE] shifted/padded copies of x."""
    xp = np.zeros((B, IN_C, D + 1, H + 1, W + 1), dtype=np.float32)
    xp[:, :, :D, :H, :W] = x
    S = np.empty((B, D, 2, 2, 2, IN_C, H, W), dtype=X_NP_DT)
    for od in range(2):
        for oh in range(2):
            for ow in range(2):
                # [B, cin, D, H, W] -> [B, D, cin, H, W]
                sl = xp[:, :, od:od + D, oh:oh + H, ow:ow + W]
                S[:, :, od, oh, ow] = sl.transpose(0, 2, 1, 3, 4).astype(X_NP_DT)
    return S.reshape(B, D, 128, PLANE)


def build_kernel():
    from concourse import bass, bacc, mybir, tile

    f32 = mybir.dt.float32
    bf16 = mybir.dt.bfloat16
    x_dt = bf16 if X_NP_DT == ml_dtypes.bfloat16 else f32
    w_dt = bf16 if W_NP_DT == ml_dtypes.bfloat16 else f32
    Alu = mybir.AluOpType
    Act = mybir.ActivationFunctionType
    Ax = mybir.AxisListType

    nc = bacc.Bacc("TRN2", target_bir_lowering=False, debug=False,
                   num_devices=N_CORES)

    xs_h = nc.declare_dram_parameter("xs", [B_PER_CORE, D, 128, PLANE], x_dt,
                                     isOutput=False)
    wr_h = nc.declare_dram_parameter("wr", [128, 256], w_dt, isOutput=False)
    bias_h = nc.declare_dram_parameter("biasrep", [128, 1024], f32,
                                       isOutput=False)
    sub_h = nc.declare_dram_parameter("subrep", [128, 1024], f32,
                                      isOutput=False)
    y_h = nc.declare_dram_parameter("y", [B_PER_CORE, D, PLANE], f32,
                                    isOutput=True)

    with tile.TileContext(nc) as tc:
        with (
            tc.tile_pool(name="const", bufs=1) as constp,
            tc.tile_pool(name="xslab", bufs=3) as xpool,
            tc.tile_pool(name="psum", bufs=4, space="PSUM") as psump,
            tc.tile_pool(name="pooled", bufs=2) as plp,
            tc.tile_pool(name="pb", bufs=2) as pbp,
            tc.tile_pool(name="exp", bufs=2) as ep,
            tc.tile_pool(name="zr", bufs=2) as zp,
            tc.tile_pool(name="sm", bufs=2) as smp,
            tc.tile_pool(name="vv", bufs=2) as vp,
            tc.tile_pool(name="mm", bufs=2) as mmp,
            tc.tile_pool(name="ext", bufs=2) as extp,
            tc.tile_pool(name="sil", bufs=2) as silp,
            tc.tile_pool(name="ost", bufs=2) as ostp,
        ):
            wr = constp.tile([128, 256], w_dt)
            nc.sync.dma_start(wr[:], wr_h[:, :])
            biasrep = constp.tile([128, 1024], f32)
            nc.sync.dma_start(biasrep[:], bias_h[:, :])
            subrep = constp.tile([128, 1024], f32)
            nc.sync.dma_start(subrep[:], sub_h[:, :])

            def emit_group(slab, pooled, g):
                """classmax for one 16-block group -> pooled[:, g]."""
                psum = psump.tile([128, GRP, 128], f32, tag="ps")
                for k in range(GRP):
                    blk = (g * GRP + k) * BLK
                    nc.tensor.matmul(
                        psum[:, k, :], slab[:, blk:blk + BLK], wr[:, 0:128],
                        start=True, stop=True)
                # pairmax = B + relu(D): relu in place (PE's has_written
                # bits survive the ScalarE write), then accumulate the
                # B-half matmuls on top.
                nc.scalar.activation(
                    psum[:].rearrange("p k c -> p (k c)"),
                    psum[:].rearrange("p k c -> p (k c)"), Act.Relu)
                for k in range(GRP):
                    blk = (g * GRP + k) * BLK
                    nc.tensor.matmul(
                        psum[:, k, :], slab[:, blk:blk + BLK], wr[:, 128:256],
                        start=False, stop=True, skip_group_check=True)
                # pooled[c] = max over the 4 (ph,pw) pair-maxes
                nc.vector.tensor_reduce(
                    pooled[:, g],
                    psum[:].rearrange("p k (c j) -> p k c j", c=32, j=4),
                    axis=Ax.X, op=Alu.max)

            def emit_tail_a(pend):
                """softmax numerator/denominator for a finished plane."""
                pooled, ext, col = pend
                pl1 = pooled[:].rearrange("p g k c -> p (g k c)")
                pb = pbp.tile([128, 1024], f32, tag="pb")
                nc.gpsimd.tensor_tensor(pb[:], pl1, biasrep[:], Alu.add)
                E = ep.tile([128, 32, 32], f32, tag="E")
                nc.scalar.activation(
                    E[:].rearrange("p a b -> p (a b)"), pb[:], Act.Exp)
                # Z = sum_c E: L1+L2+L3 on gpsimd, final reduce-4 on DVE
                e1 = mmp.tile([128, 32, 16], f32, tag="e1")
                nc.gpsimd.tensor_tensor(e1[:], E[:, :, 0:16], E[:, :, 16:32],
                                        Alu.add)
                e2 = mmp.tile([128, 32, 8], f32, tag="e2")
                nc.gpsimd.tensor_tensor(e2[:], e1[:, :, 0:8], e1[:, :, 8:16],
                                        Alu.add)
                e3 = mmp.tile([128, 32, 4], f32, tag="e3")
                nc.gpsimd.tensor_tensor(e3[:], e2[:, :, 0:4], e2[:, :, 4:8],
                                        Alu.add)
                Z = zp.tile([128, 32], f32, tag="Z")
                nc.vector.tensor_reduce(Z[:], e3[:], axis=Ax.X, op=Alu.add)
                R = zp.tile([128, 32], f32, tag="R")
                nc.vector.reciprocal(R[:], Z[:])
                return E, R

            def emit_tail_b(pend, E, R):
                """normalize, subtract, channel max/min extremes."""
                pooled, ext, col = pend
                sm = smp.tile([128, 32, 32], f32, tag="sm")
                nc.gpsimd.tensor_tensor(
                    sm[:], E[:],
                    R[:].unsqueeze(2).broadcast_to([128, 32, 32]), Alu.mult)
                # v in bf16 so the DVE max/min trees run in 2x mode
                # (Pool TT does not support max/min per walrus codegen)
                v = vp.tile([128, 32, 32], bf16, tag="v")
                nc.gpsimd.tensor_tensor(
                    v[:], sm[:],
                    subrep[:].rearrange("p (a b) -> p a b", a=32, b=32),
                    Alu.subtract)
                m1 = mmp.tile([128, 32, 16], bf16, tag="m1")
                nc.vector.tensor_tensor(m1[:], v[:, :, 0:16], v[:, :, 16:32],
                                        Alu.max)
                m2 = mmp.tile([128, 32, 8], bf16, tag="m2")
                nc.vector.tensor_tensor(m2[:], m1[:, :, 0:8], m1[:, :, 8:16],
                                        Alu.max)
                nc.vector.tensor_reduce(
                    ext[:, 0, col:col + 32], m2[:], axis=Ax.X, op=Alu.max)
                # min(a,b) = (a+b) - max(a,b): L1 of the min tree rides Pool
                s1 = mmp.tile([128, 32, 16], f32, tag="s1")
                nc.gpsimd.tensor_tensor(s1[:], v[:, :, 0:16], v[:, :, 16:32],
                                        Alu.add)
                n1 = mmp.tile([128, 32, 16], bf16, tag="n1")
                nc.gpsimd.tensor_tensor(n1[:], s1[:], m1[:], Alu.subtract)
                n2 = mmp.tile([128, 32, 8], bf16, tag="n2")
                nc.vector.tensor_tensor(n2[:], n1[:, :, 0:8], n1[:, :, 8:16],
                                        Alu.min)
                nc.vector.tensor_reduce(
                    ext[:, 1, col:col + 32], n2[:], axis=Ax.X, op=Alu.min)

            def emit_b_final(b, ext):
                """silu on the per-b extremes, final max, output DMA."""
                sil = silp.tile([128, 2, D * BLKS_PER_PLANE], f32, tag="sil")
                nc.scalar.activation(
                    sil[:].rearrange("p a b -> p (a b)"),
                    ext[:].rearrange("p a b -> p (a b)"), Act.Silu)
                ost = ostp.tile([128, D * BLKS_PER_PLANE], f32, tag="ost")
                nc.vector.tensor_tensor(ost[:], sil[:, 0, :], sil[:, 1, :],
                                        Alu.max)
                # ost[p, (d, blk)] ; plane pos = blk*128 + p
                nc.sync.dma_start(
                    y_h[b].flatten().rearrange(
                        "(dd blk p) -> p dd blk",
                        dd=D, blk=BLKS_PER_PLANE, p=BLK),
                    ost[:].rearrange("p (dd blk) -> p dd blk",
                                     dd=D, blk=BLKS_PER_PLANE))

            # Software-pipelined by one plane: the previous plane's tail is
            # interleaved between the current plane's two group reduces so
            # the DVE never waits on the Pool mult/sub chain.
            exts = []
            for b in range(B_PER_CORE):
                ext_b = extp.tile([128, 2, D * BLKS_PER_PLANE], f32,
                                  tag=f"ext{b}", name=f"ext{b}")
                exts.append(ext_b)
            pend = None
            done_b = None
            for b in range(B_PER_CORE):
                for d in range(D):
                    slab = xpool.tile([128, PLANE], x_dt, tag="slab")
                    # per-group DMA slices so the first matmuls start sooner
                    for g in range(GRPS_PER_PLANE):
                        c0 = g * GRP * BLK
                        nc.sync.dma_start(slab[:, c0:c0 + GRP * BLK],
                                          xs_h[b, d, :, c0:c0 + GRP * BLK])
                    pooled = plp.tile([128, GRPS_PER_PLANE, GRP, 32], f32,
                                      tag="pooled")
                    half = GRPS_PER_PLANE // 2
                    for g in range(half):
                        emit_group(slab, pooled, g)
                    if pend is not None:
                        ER = emit_tail_a(pend)
                    for g in range(half, GRPS_PER_PLANE):
                        emit_group(slab, pooled, g)
                    if pend is not None:
                        emit_tail_b(pend, *ER)
                        if done_b is not None:
                            emit_b_final(*done_b)
                            done_b = None
                    pend = (pooled, exts[b], d * BLKS_PER_PLANE)
                    if d == D - 1:
                        done_b = (b, exts[b])
            # drain: last plane's tail + final b output
            ER = emit_tail_a(pend)
            emit_tail_b(pend, *ER)
            emit_b_final(*done_b)

    nc.compile()
    return nc


def _get_nc():
    if "nc" not in _COMPILED:
        _COMPILED["nc"] = build_kernel()
    return _COMPILED["nc"]


LAST_EXEC_NS = None


def build_in_maps(xs, wr, bias, subtract):
    # biasrep/subrep: [128, 1024] pattern (32 blocks x 32 ch), f32
    biasrep = np.tile(bias[None, None, :], (128, 32, 1)).reshape(
        128, 1024).astype(np.float32)
    subrep = np.tile(subtract[None, None, :], (128, 32, 1)).reshape(
        128, 1024).astype(np.float32)

    in_maps = []
    for c in range(N_CORES):
        in_maps.append({
            "xs": np.ascontiguousarray(xs[c * B_PER_CORE:(c + 1) * B_PER_CORE]),
            "wr": wr,
            "biasrep": biasrep,
            "subrep": subrep,
        })
    return in_maps


def kernel(x, weight, bias, subtract):
    from concourse.bass_utils import run_bass_kernel_spmd

    x = np.asarray(x, dtype=np.float32)
    weight = np.asarray(weight, dtype=np.float32)
    bias = np.asarray(bias, dtype=np.float32)
    subtract = np.asarray(subtract, dtype=np.float32)

    nc = _get_nc()

    xs = build_xstack(x)                      # [B, D, 128, PLANE]
    wr = build_wrhs(weight).astype(W_NP_DT)   # [128, 256]
    in_maps = build_in_maps(xs, wr, bias, subtract)

    res = run_bass_kernel_spmd(nc, in_maps, core_ids=list(range(N_CORES)))
    outs = [res.results[c]["y"].reshape(B_PER_CORE, D, H, W)
            for c in range(N_CORES)]
    return np.concatenate(outs, axis=0)


# revision 18
# speedup vs baseline: 18.7406x; 5.9246x over previous
"""Trainium2 Bass kernel for nn_ModelNew_3556232921999.

Pipeline: ConvTranspose3d(16->32, k=3, s=2, p=1, op=1) -> MaxPool3d(2)
          -> softmax(ch) -> subtract -> swish -> max(ch)

Key algebraic structure exploited:
  * convT(stride 2) + maxpool(2,2) => output spatial == input spatial, and the
    pool window {2m, 2m+1}^3 corresponds to the 8 parity classes of the convT.
    Each parity class is a small conv over x with taps at offsets {0,1}^3.
    pooled[c, m] = max over the 8 classes (+ bias, which commutes with max).
  * All parity classes for one position come out of matmuls with
      lhsT = x-stack block [K=128 = (od,oh,ow,cin), M=128 positions]
    so channel/class dims land on the FREE axis and the softmax reductions
    are free-dim reductions.
  * Pair-max via ReLU identity: max(a,b) = b + relu(a-b). The pd-pair of
    classes is computed as D = x@(W0-W1) (one matmul), relu'd IN PLACE in
    PSUM by the scalar engine (PE's has_written bits survive engine writes),
    then B = x@W1 is accumulated on top with start=False. One segmented
    tensor_reduce(max) over the remaining 4 (ph,pw) candidates then yields
    the pooled value -- no DVE max tree.
  * swish/silu is quasiconvex => max_c silu(v_c) = max(silu(max_c v),
    silu(min_c v)), so silu runs on 2 values per position, not 32.

Sharding: data-parallel over batch B=16 -> 2 per core x 8 cores.
"""

import os
import sys

sys.path.insert(0, "/opt/trn_rl_repo")

import numpy as np
import ml_dtypes

# ---------------------------------------------------------------- constants
IN_C, OUT_C, K, STRIDE, PAD, OUT_PAD = 16, 32, 3, 2, 1, 1
B, D, H, W = 16, 16, 64, 64
N_CORES = 8
B_PER_CORE = B // N_CORES  # 2

PLANE = H * W            # 4096 positions per (b, d) plane
BLK = 128                # positions per matmul block
BLKS_PER_PLANE = PLANE // BLK      # 32
GRP = 8                  # matmul blocks per psum group (2-bank tile)
GRPS_PER_PLANE = BLKS_PER_PLANE // GRP  # 2

X_NP_DT = ml_dtypes.bfloat16   # x-stack storage dtype (DMA volume)
W_NP_DT = ml_dtypes.bfloat16   # conv weight dtype

_COMPILED = {}


def _tap(o, p):
    """Kernel tap index used by parity class p at window offset o, or None."""
    if p == 0:
        return 1 if o == 0 else None
    return 2 if o == 0 else 0


def build_w8(weight):
    """[128 rows=(od,oh,ow,cin), 2,2,2,32 cols=(pd,ph,pw,c)] conv matrix."""
    wr = np.zeros((2, 2, 2, IN_C, 2, 2, 2, OUT_C), dtype=np.float32)
    for od in range(2):
        for oh in range(2):
            for ow in range(2):
                for pd in range(2):
                    kd = _tap(od, pd)
                    if kd is None:
                        continue
                    for ph in range(2):
                        kh = _tap(oh, ph)
                        if kh is None:
                            continue
                        for pw in range(2):
                            kw = _tap(ow, pw)
                            if kw is None:
                                continue
                            # weight: [cin, cout, kd, kh, kw]
                            wr[od, oh, ow, :, pd, ph, pw, :] = weight[:, :, kd, kh, kw]
    return wr.reshape(128, 2, 2, 2, OUT_C)


def build_wrhs(weight):
    """[128, 256] = [D-half | B-half], col within half = (c, j=(ph,pw)).

    D = W_pd0 - W_pd1 (pair differences), B = W_pd1 (pair base), so that
    pairmax = B + relu(D)."""
    w8 = build_w8(weight)                      # [128, pd, ph, pw, c]
    # [128, ph, pw, c] -> order cols (c, j): transpose to [128, c, ph, pw]
    w0 = w8[:, 0].transpose(0, 3, 1, 2).reshape(128, 128)
    w1 = w8[:, 1].transpose(0, 3, 1, 2).reshape(128, 128)
    return np.concatenate([w0 - w1, w1], axis=1)  # [128, 256]


def build_xstack(x):
    """[B, D, 128 rows=(od,oh,ow,cin), PLANE] shifted/padded copies of x."""
    xp = np.zeros((B, IN_C, D + 1, H + 1, W + 1), dtype=np.float32)
    xp[:, :, :D, :H, :W] = x
    S = np.empty((B, D, 2, 2, 2, IN_C, H, W), dtype=X_NP_DT)
    for od in range(2):
        for oh in range(2):
            for ow in range(2):
                # [B, cin, D, H, W] -> [B, D, cin, H, W]
                sl = xp[:, :, od:od + D, oh:oh + H, ow:ow + W]
                S[:, :, od, oh, ow] = sl.transpose(0, 2, 1, 3, 4).astype(X_NP_DT)
    return S.reshape(B, D, 128, PLANE)


def build_kernel(passes=1):
    from concourse import bass, bacc, mybir, tile

    f32 = mybir.dt.float32
    bf16 = mybir.dt.bfloat16
    x_dt = bf16 if X_NP_DT == ml_dtypes.bfloat16 else f32
    w_dt = bf16 if W_NP_DT == ml_dtypes.bfloat16 else f32
    Alu = mybir.AluOpType
    Act = mybir.ActivationFunctionType
    Ax = mybir.AxisListType

    nc = bacc.Bacc("TRN2", target_bir_lowering=False, debug=False,
                   num_devices=N_CORES)

    xs_h = nc.declare_dram_parameter("xs", [B_PER_CORE, D, 128, PLANE], x_dt,
                                     isOutput=False)
    wr_h = nc.declare_dram_parameter("wr", [128, 256], w_dt, isOutput=False)
    bias_h = nc.declare_dram_parameter("biasrep", [128, 1024], f32,
                                       isOutput=False)
    sub_h = nc.declare_dram_parameter("subrep", [128, 1024], f32,
                                      isOutput=False)
    y_h = nc.declare_dram_parameter("y", [B_PER_CORE, D, PLANE], f32,
                                    isOutput=True)

    with tile.TileContext(nc) as tc:
        with (
            tc.tile_pool(name="const", bufs=1) as constp,
            tc.tile_pool(name="xslab", bufs=4) as xpool,
            tc.tile_pool(name="psum", bufs=4, space="PSUM") as psump,
            tc.tile_pool(name="pooled", bufs=3) as plp,
            tc.tile_pool(name="pb", bufs=2) as pbp,
            tc.tile_pool(name="exp", bufs=2) as ep,
            tc.tile_pool(name="zr", bufs=2) as zp,
            tc.tile_pool(name="sm", bufs=3) as smp,
            tc.tile_pool(name="vv", bufs=3) as vp,
            tc.tile_pool(name="mm", bufs=3) as mmp,
            tc.tile_pool(name="ext", bufs=2) as extp,
            tc.tile_pool(name="sil", bufs=2) as silp,
            tc.tile_pool(name="ost", bufs=2) as ostp,
        ):
            wr = constp.tile([128, 256], w_dt)
            nc.sync.dma_start(wr[:], wr_h[:, :])
            biasrep = constp.tile([128, 1024], f32)
            nc.sync.dma_start(biasrep[:], bias_h[:, :])
            subrep = constp.tile([128, 1024], f32)
            nc.sync.dma_start(subrep[:], sub_h[:, :])

            def emit_group(slab, pooled, g):
                """classmax for one 16-block group -> pooled[:, g]."""
                psum = psump.tile([128, GRP, 128], f32, tag="ps")
                for k in range(GRP):
                    blk = (g * GRP + k) * BLK
                    nc.tensor.matmul(
                        psum[:, k, :], slab[:, blk:blk + BLK], wr[:, 0:128],
                        start=True, stop=True)
                # pairmax = B + relu(D): relu in place (PE's has_written
                # bits survive the ScalarE write), then accumulate the
                # B-half matmuls on top.
                nc.scalar.activation(
                    psum[:].rearrange("p k c -> p (k c)"),
                    psum[:].rearrange("p k c -> p (k c)"), Act.Relu)
                for k in range(GRP):
                    blk = (g * GRP + k) * BLK
                    nc.tensor.matmul(
                        psum[:, k, :], slab[:, blk:blk + BLK], wr[:, 128:256],
                        start=False, stop=True, skip_group_check=True)
                # pooled[c] = max over the 4 (ph,pw) pair-maxes
                nc.vector.tensor_reduce(
                    pooled[:, g],
                    psum[:].rearrange("p k (c j) -> p k c j", c=32, j=4),
                    axis=Ax.X, op=Alu.max)

            def emit_tail_a(pend):
                """softmax numerator/denominator for a finished plane."""
                pooled, ext, col = pend
                pl1 = pooled[:].rearrange("p g k c -> p (g k c)")
                pb = pbp.tile([128, 1024], f32, tag="pb")
                nc.gpsimd.tensor_tensor(pb[:], pl1, biasrep[:], Alu.add)
                E = ep.tile([128, 32, 32], f32, tag="E")
                nc.scalar.activation(
                    E[:].rearrange("p a b -> p (a b)"), pb[:], Act.Exp)
                # Z = sum_c E: L1+L2+L3 on gpsimd, final reduce-4 on DVE
                e1 = mmp.tile([128, 32, 16], f32, tag="e1")
                nc.gpsimd.tensor_tensor(e1[:], E[:, :, 0:16], E[:, :, 16:32],
                                        Alu.add)
                e2 = mmp.tile([128, 32, 8], f32, tag="e2")
                nc.gpsimd.tensor_tensor(e2[:], e1[:, :, 0:8], e1[:, :, 8:16],
                                        Alu.add)
                e3 = mmp.tile([128, 32, 4], f32, tag="e3")
                nc.gpsimd.tensor_tensor(e3[:], e2[:, :, 0:4], e2[:, :, 4:8],
                                        Alu.add)
                Z = zp.tile([128, 32], f32, tag="Z")
                nc.vector.tensor_reduce(Z[:], e3[:], axis=Ax.X, op=Alu.add)
                R = zp.tile([128, 32], f32, tag="R")
                nc.vector.reciprocal(R[:], Z[:])
                return E, R

            def emit_tail_b(pend, E, R):
                """normalize, subtract, channel max/min extremes."""
                pooled, ext, col = pend
                sm = smp.tile([128, 32, 32], f32, tag="sm")
                nc.gpsimd.tensor_tensor(
                    sm[:], E[:],
                    R[:].unsqueeze(2).broadcast_to([128, 32, 32]), Alu.mult)
                # v in bf16 so the DVE max/min trees run in 2x mode
                # (Pool TT does not support max/min per walrus codegen)
                v = vp.tile([128, 32, 32], bf16, tag="v")
                nc.gpsimd.tensor_tensor(
                    v[:], sm[:],
                    subrep[:].rearrange("p (a b) -> p a b", a=32, b=32),
                    Alu.subtract)
                m1 = mmp.tile([128, 32, 16], bf16, tag="m1")
                nc.vector.tensor_tensor(m1[:], v[:, :, 0:16], v[:, :, 16:32],
                                        Alu.max)
                m2 = mmp.tile([128, 32, 8], bf16, tag="m2")
                nc.vector.tensor_tensor(m2[:], m1[:, :, 0:8], m1[:, :, 8:16],
                                        Alu.max)
                nc.vector.tensor_reduce(
                    ext[:, 0, col:col + 32], m2[:], axis=Ax.X, op=Alu.max)
                # min(a,b) = (a+b) - max(a,b): L1 of the min tree rides Pool
                s1 = mmp.tile([128, 32, 16], f32, tag="s1")
                nc.gpsimd.tensor_tensor(s1[:], v[:, :, 0:16], v[:, :, 16:32],
                                        Alu.add)
                n1 = mmp.tile([128, 32, 16], bf16, tag="n1")
                nc.gpsimd.tensor_tensor(n1[:], s1[:], m1[:], Alu.subtract)
                n2 = mmp.tile([128, 32, 8], bf16, tag="n2")
                nc.vector.tensor_tensor(n2[:], n1[:, :, 0:8], n1[:, :, 8:16],
                                        Alu.min)
                nc.vector.tensor_reduce(
                    ext[:, 1, col:col + 32], n2[:], axis=Ax.X, op=Alu.min)

            def emit_b_final(b, ext):
                """silu on the per-b extremes, final max, output DMA."""
                sil = silp.tile([128, 2, D * BLKS_PER_PLANE], f32, tag="sil")
                nc.scalar.activation(
                    sil[:].rearrange("p a b -> p (a b)"),
                    ext[:].rearrange("p a b -> p (a b)"), Act.Silu)
                ost = ostp.tile([128, D * BLKS_PER_PLANE], f32, tag="ost")
                nc.vector.tensor_tensor(ost[:], sil[:, 0, :], sil[:, 1, :],
                                        Alu.max)
                # ost[p, (d, blk)] ; plane pos = blk*128 + p
                nc.sync.dma_start(
                    y_h[b].flatten().rearrange(
                        "(dd blk p) -> p dd blk",
                        dd=D, blk=BLKS_PER_PLANE, p=BLK),
                    ost[:].rearrange("p (dd blk) -> p dd blk",
                                     dd=D, blk=BLKS_PER_PLANE))

            # Software-pipelined by one plane: the previous plane's tail is
            # interleaved between the current plane's two group reduces so
            # the DVE never waits on the Pool mult/sub chain.
            exts = []
            for b in range(B_PER_CORE):
                ext_b = extp.tile([128, 2, D * BLKS_PER_PLANE], f32,
                                  tag=f"ext{b}", name=f"ext{b}")
                exts.append(ext_b)
            pend = None
            done_b = None
            for b_outer in range(passes * B_PER_CORE):
                b = b_outer % B_PER_CORE
                for d in range(D):
                    slab = xpool.tile([128, PLANE], x_dt, tag="slab")
                    # per-group DMA slices so the first matmuls start sooner
                    for g in range(GRPS_PER_PLANE):
                        c0 = g * GRP * BLK
                        nc.sync.dma_start(slab[:, c0:c0 + GRP * BLK],
                                          xs_h[b, d, :, c0:c0 + GRP * BLK])
                    pooled = plp.tile([128, GRPS_PER_PLANE, GRP, 32], f32,
                                      tag="pooled")
                    half = GRPS_PER_PLANE // 2
                    for g in range(half):
                        emit_group(slab, pooled, g)
                    if pend is not None:
                        ER = emit_tail_a(pend)
                    for g in range(half, GRPS_PER_PLANE):
                        emit_group(slab, pooled, g)
                    if pend is not None:
                        emit_tail_b(pend, *ER)
                        if done_b is not None:
                            emit_b_final(*done_b)
                            done_b = None
                    pend = (pooled, exts[b], d * BLKS_PER_PLANE)
                    if d == D - 1:
                        done_b = (b, exts[b])
            # drain: last plane's tail + final b output
            ER = emit_tail_a(pend)
            emit_tail_b(pend, *ER)
            emit_b_final(*done_b)

    nc.compile()
    return nc


def _get_nc(passes=1):
    key = f"nc{passes}"
    if key not in _COMPILED:
        _COMPILED[key] = build_kernel(passes)
    return _COMPILED[key]


LAST_EXEC_NS = None


def build_in_maps(xs, wr, bias, subtract):
    # biasrep/subrep: [128, 1024] pattern (32 blocks x 32 ch), f32
    biasrep = np.tile(bias[None, None, :], (128, 32, 1)).reshape(
        128, 1024).astype(np.float32)
    subrep = np.tile(subtract[None, None, :], (128, 32, 1)).reshape(
        128, 1024).astype(np.float32)

    in_maps = []
    for c in range(N_CORES):
        in_maps.append({
            "xs": np.ascontiguousarray(xs[c * B_PER_CORE:(c + 1) * B_PER_CORE]),
            "wr": wr,
            "biasrep": biasrep,
            "subrep": subrep,
        })
    return in_maps


def kernel(x, weight, bias, subtract):
    from concourse.bass_utils import run_bass_kernel_spmd

    x = np.asarray(x, dtype=np.float32)
    weight = np.asarray(weight, dtype=np.float32)
    bias = np.asarray(bias, dtype=np.float32)
    subtract = np.asarray(subtract, dtype=np.float32)

    nc = _get_nc()

    xs = build_xstack(x)                      # [B, D, 128, PLANE]
    wr = build_wrhs(weight).astype(W_NP_DT)   # [128, 256]
    in_maps = build_in_maps(xs, wr, bias, subtract)

    res = run_bass_kernel_spmd(nc, in_maps, core_ids=list(range(N_CORES)))
    outs = [res.results[c]["y"].reshape(B_PER_CORE, D, H, W)
            for c in range(N_CORES)]
    return np.concatenate(outs, axis=0)


# revision 19
# speedup vs baseline: 19.2362x; 1.0264x over previous
"""Trainium2 Bass kernel for nn_ModelNew_3556232921999.

Pipeline: ConvTranspose3d(16->32, k=3, s=2, p=1, op=1) -> MaxPool3d(2)
          -> softmax(ch) -> subtract -> swish -> max(ch)

Key algebraic structure exploited:
  * convT(stride 2) + maxpool(2,2) => output spatial == input spatial, and the
    pool window {2m, 2m+1}^3 corresponds to the 8 parity classes of the convT.
    Each parity class is a small conv over x with taps at offsets {0,1}^3.
    pooled[c, m] = max over the 8 classes (+ bias, which commutes with max).
  * All parity classes for one position come out of matmuls with
      lhsT = x-stack block [K=128 = (od,oh,ow,cin), M=128 positions]
    so channel/class dims land on the FREE axis and the softmax reductions
    are free-dim reductions.
  * Pair-max via ReLU identity: max(a,b) = b + relu(a-b). The pd-pair of
    classes is computed as D = x@(W0-W1) (one matmul), relu'd IN PLACE in
    PSUM by the scalar engine (PE's has_written bits survive engine writes),
    then B = x@W1 is accumulated on top with start=False. One segmented
    tensor_reduce(max) over the remaining 4 (ph,pw) candidates then yields
    the pooled value -- no DVE max tree.
  * swish/silu is quasiconvex => max_c silu(v_c) = max(silu(max_c v),
    silu(min_c v)), so silu runs on 2 values per position, not 32.

Sharding: data-parallel over batch B=16 -> 2 per core x 8 cores.
"""

import os
import sys

sys.path.insert(0, "/opt/trn_rl_repo")

import numpy as np
import ml_dtypes

# ---------------------------------------------------------------- constants
IN_C, OUT_C, K, STRIDE, PAD, OUT_PAD = 16, 32, 3, 2, 1, 1
B, D, H, W = 16, 16, 64, 64
N_CORES = 8
B_PER_CORE = B // N_CORES  # 2

PLANE = H * W            # 4096 positions per (b, d) plane
BLK = 128                # positions per matmul block
BLKS_PER_PLANE = PLANE // BLK      # 32
GRP = 8                  # matmul blocks per psum group (2-bank tile)
GRPS_PER_PLANE = BLKS_PER_PLANE // GRP  # 2

X_NP_DT = ml_dtypes.bfloat16   # x-stack storage dtype (DMA volume)
W_NP_DT = ml_dtypes.bfloat16   # conv weight dtype

_COMPILED = {}


def _tap(o, p):
    """Kernel tap index used by parity class p at window offset o, or None."""
    if p == 0:
        return 1 if o == 0 else None
    return 2 if o == 0 else 0


def build_w8(weight):
    """[128 rows=(od,oh,ow,cin), 2,2,2,32 cols=(pd,ph,pw,c)] conv matrix."""
    wr = np.zeros((2, 2, 2, IN_C, 2, 2, 2, OUT_C), dtype=np.float32)
    for od in range(2):
        for oh in range(2):
            for ow in range(2):
                for pd in range(2):
                    kd = _tap(od, pd)
                    if kd is None:
                        continue
                    for ph in range(2):
                        kh = _tap(oh, ph)
                        if kh is None:
                            continue
                        for pw in range(2):
                            kw = _tap(ow, pw)
                            if kw is None:
                                continue
                            # weight: [cin, cout, kd, kh, kw]
                            wr[od, oh, ow, :, pd, ph, pw, :] = weight[:, :, kd, kh, kw]
    return wr.reshape(128, 2, 2, 2, OUT_C)


def build_wrhs(weight):
    """[128, 256] = [D-half | B-half], col within half = (c, j=(ph,pw)).

    D = W_pd0 - W_pd1 (pair differences), B = W_pd1 (pair base), so that
    pairmax = B + relu(D)."""
    w8 = build_w8(weight)                      # [128, pd, ph, pw, c]
    # [128, ph, pw, c] -> order cols (c, j): transpose to [128, c, ph, pw]
    w0 = w8[:, 0].transpose(0, 3, 1, 2).reshape(128, 128)
    w1 = w8[:, 1].transpose(0, 3, 1, 2).reshape(128, 128)
    return np.concatenate([w0 - w1, w1], axis=1)  # [128, 256]


def build_xstack(x):
    """[B, D, 128 rows=(od,oh,ow,cin), PLANE] shifted/padded copies of x."""
    xp = np.zeros((B, IN_C, D + 1, H + 1, W + 1), dtype=np.float32)
    xp[:, :, :D, :H, :W] = x
    S = np.empty((B, D, 2, 2, 2, IN_C, H, W), dtype=X_NP_DT)
    for od in range(2):
        for oh in range(2):
            for ow in range(2):
                # [B, cin, D, H, W] -> [B, D, cin, H, W]
                sl = xp[:, :, od:od + D, oh:oh + H, ow:ow + W]
                S[:, :, od, oh, ow] = sl.transpose(0, 2, 1, 3, 4).astype(X_NP_DT)
    return S.reshape(B, D, 128, PLANE)


def build_kernel(passes=1):
    from concourse import bass, bacc, mybir, tile

    f32 = mybir.dt.float32
    bf16 = mybir.dt.bfloat16
    x_dt = bf16 if X_NP_DT == ml_dtypes.bfloat16 else f32
    w_dt = bf16 if W_NP_DT == ml_dtypes.bfloat16 else f32
    Alu = mybir.AluOpType
    Act = mybir.ActivationFunctionType
    Ax = mybir.AxisListType

    nc = bacc.Bacc("TRN2", target_bir_lowering=False, debug=False,
                   num_devices=N_CORES)

    xs_h = nc.declare_dram_parameter("xs", [B_PER_CORE, D, 128, PLANE], x_dt,
                                     isOutput=False)
    wr_h = nc.declare_dram_parameter("wr", [128, 256], w_dt, isOutput=False)
    bias_h = nc.declare_dram_parameter("biasrep", [128, 1024], f32,
                                       isOutput=False)
    sub_h = nc.declare_dram_parameter("subrep", [128, 1024], f32,
                                      isOutput=False)
    y_h = nc.declare_dram_parameter("y", [B_PER_CORE, D, PLANE], f32,
                                    isOutput=True)

    with tile.TileContext(nc) as tc:
        with (
            tc.tile_pool(name="const", bufs=1) as constp,
            tc.tile_pool(name="xslab", bufs=4) as xpool,
            tc.tile_pool(name="psum", bufs=4, space="PSUM") as psump,
            tc.tile_pool(name="pooled", bufs=3) as plp,
            tc.tile_pool(name="pb", bufs=2) as pbp,
            tc.tile_pool(name="exp", bufs=2) as ep,
            tc.tile_pool(name="zr", bufs=2) as zp,
            tc.tile_pool(name="sm", bufs=3) as smp,
            tc.tile_pool(name="vv", bufs=3) as vp,
            tc.tile_pool(name="mm", bufs=3) as mmp,
            tc.tile_pool(name="ext", bufs=2) as extp,
            tc.tile_pool(name="sil", bufs=2) as silp,
            tc.tile_pool(name="ost", bufs=2) as ostp,
        ):
            wr = constp.tile([128, 256], w_dt)
            nc.sync.dma_start(wr[:], wr_h[:, :])
            biasrep = constp.tile([128, 1024], f32)
            nc.sync.dma_start(biasrep[:], bias_h[:, :])
            subrep = constp.tile([128, 1024], f32)
            nc.sync.dma_start(subrep[:], sub_h[:, :])

            def emit_group(slab, pooled, g):
                """classmax for one 16-block group -> pooled[:, g]."""
                psum = psump.tile([128, GRP, 128], f32, tag="ps")
                for k in range(GRP):
                    blk = (g * GRP + k) * BLK
                    nc.tensor.matmul(
                        psum[:, k, :], slab[:, blk:blk + BLK], wr[:, 0:128],
                        start=True, stop=True)
                # pairmax = B + relu(D): relu in place (PE's has_written
                # bits survive the ScalarE write), then accumulate the
                # B-half matmuls on top.
                nc.scalar.activation(
                    psum[:].rearrange("p k c -> p (k c)"),
                    psum[:].rearrange("p k c -> p (k c)"), Act.Relu)
                for k in range(GRP):
                    blk = (g * GRP + k) * BLK
                    nc.tensor.matmul(
                        psum[:, k, :], slab[:, blk:blk + BLK], wr[:, 128:256],
                        start=False, stop=True, skip_group_check=True)
                # pooled[c] = max over the 4 (ph,pw) pair-maxes
                nc.vector.tensor_reduce(
                    pooled[:, g],
                    psum[:].rearrange("p k (c j) -> p k c j", c=32, j=4),
                    axis=Ax.X, op=Alu.max)

            def emit_tail_a(pend):
                """softmax numerator/denominator for a finished plane."""
                pooled, ext, col = pend
                pl1 = pooled[:].rearrange("p g k c -> p (g k c)")
                pb = pbp.tile([128, 1024], f32, tag="pb")
                nc.gpsimd.tensor_tensor(pb[:], pl1, biasrep[:], Alu.add)
                E = ep.tile([128, 32, 32], f32, tag="E")
                nc.scalar.activation(
                    E[:].rearrange("p a b -> p (a b)"), pb[:], Act.Exp)
                # Z = sum_c E: L1+L2+L3 on gpsimd, final reduce-4 on DVE
                e1 = mmp.tile([128, 32, 16], f32, tag="e1")
                nc.gpsimd.tensor_tensor(e1[:], E[:, :, 0:16], E[:, :, 16:32],
                                        Alu.add)
                e2 = mmp.tile([128, 32, 8], f32, tag="e2")
                nc.gpsimd.tensor_tensor(e2[:], e1[:, :, 0:8], e1[:, :, 8:16],
                                        Alu.add)
                e3 = mmp.tile([128, 32, 4], f32, tag="e3")
                nc.gpsimd.tensor_tensor(e3[:], e2[:, :, 0:4], e2[:, :, 4:8],
                                        Alu.add)
                Z = zp.tile([128, 32], f32, tag="Z")
                nc.vector.tensor_reduce(Z[:], e3[:], axis=Ax.X, op=Alu.add)
                R = zp.tile([128, 32], f32, tag="R")
                nc.vector.reciprocal(R[:], Z[:])
                return E, R

            def emit_tail_b(pend, E, R):
                """normalize, subtract, channel max/min extremes."""
                pooled, ext, col = pend
                sm = smp.tile([128, 32, 32], f32, tag="sm")
                nc.gpsimd.tensor_tensor(
                    sm[:], E[:],
                    R[:].unsqueeze(2).broadcast_to([128, 32, 32]), Alu.mult)
                # v in bf16 so the DVE max/min trees run in 2x mode
                # (Pool TT does not support max/min per walrus codegen)
                v = vp.tile([128, 32, 32], bf16, tag="v")
                nc.gpsimd.tensor_tensor(
                    v[:], sm[:],
                    subrep[:].rearrange("p (a b) -> p a b", a=32, b=32),
                    Alu.subtract)
                # max L1 via relu identity: Pool sub + ACT relu + Pool add
                # (keeps the first tree level off the saturated DVE)
                dd = mmp.tile([128, 32, 16], f32, tag="dd")
                nc.gpsimd.tensor_tensor(dd[:], v[:, :, 0:16], v[:, :, 16:32],
                                        Alu.subtract)
                rr = mmp.tile([128, 32, 16], f32, tag="rr")
                nc.scalar.activation(
                    rr[:].rearrange("p a b -> p (a b)"),
                    dd[:].rearrange("p a b -> p (a b)"), Act.Relu)
                m1 = mmp.tile([128, 32, 16], bf16, tag="m1")
                nc.gpsimd.tensor_tensor(m1[:], v[:, :, 16:32], rr[:], Alu.add)
                m2 = mmp.tile([128, 32, 8], bf16, tag="m2")
                nc.vector.tensor_tensor(m2[:], m1[:, :, 0:8], m1[:, :, 8:16],
                                        Alu.max)
                nc.vector.tensor_reduce(
                    ext[:, 0, col:col + 32], m2[:], axis=Ax.X, op=Alu.max)
                # min(a,b) = (a+b) - max(a,b): L1 of the min tree rides Pool
                s1 = mmp.tile([128, 32, 16], f32, tag="s1")
                nc.gpsimd.tensor_tensor(s1[:], v[:, :, 0:16], v[:, :, 16:32],
                                        Alu.add)
                n1 = mmp.tile([128, 32, 16], bf16, tag="n1")
                nc.gpsimd.tensor_tensor(n1[:], s1[:], m1[:], Alu.subtract)
                n2 = mmp.tile([128, 32, 8], bf16, tag="n2")
                nc.vector.tensor_tensor(n2[:], n1[:, :, 0:8], n1[:, :, 8:16],
                                        Alu.min)
                nc.vector.tensor_reduce(
                    ext[:, 1, col:col + 32], n2[:], axis=Ax.X, op=Alu.min)

            def emit_b_final(b, ext):
                """silu on the per-b extremes, final max, output DMA."""
                sil = silp.tile([128, 2, D * BLKS_PER_PLANE], f32, tag="sil")
                nc.scalar.activation(
                    sil[:].rearrange("p a b -> p (a b)"),
                    ext[:].rearrange("p a b -> p (a b)"), Act.Silu)
                ost = ostp.tile([128, D * BLKS_PER_PLANE], f32, tag="ost")
                nc.vector.tensor_tensor(ost[:], sil[:, 0, :], sil[:, 1, :],
                                        Alu.max)
                # ost[p, (d, blk)] ; plane pos = blk*128 + p
                nc.sync.dma_start(
                    y_h[b].flatten().rearrange(
                        "(dd blk p) -> p dd blk",
                        dd=D, blk=BLKS_PER_PLANE, p=BLK),
                    ost[:].rearrange("p (dd blk) -> p dd blk",
                                     dd=D, blk=BLKS_PER_PLANE))

            # Software-pipelined by one plane: the previous plane's tail is
            # interleaved between the current plane's two group reduces so
            # the DVE never waits on the Pool mult/sub chain.
            exts = []
            for b in range(B_PER_CORE):
                ext_b = extp.tile([128, 2, D * BLKS_PER_PLANE], f32,
                                  tag=f"ext{b}", name=f"ext{b}")
                exts.append(ext_b)
            pend = None
            done_b = None
            for b_outer in range(passes * B_PER_CORE):
                b = b_outer % B_PER_CORE
                for d in range(D):
                    slab = xpool.tile([128, PLANE], x_dt, tag="slab")
                    # per-group DMA slices so the first matmuls start sooner
                    for g in range(GRPS_PER_PLANE):
                        c0 = g * GRP * BLK
                        nc.sync.dma_start(slab[:, c0:c0 + GRP * BLK],
                                          xs_h[b, d, :, c0:c0 + GRP * BLK])
                    pooled = plp.tile([128, GRPS_PER_PLANE, GRP, 32], f32,
                                      tag="pooled")
                    half = GRPS_PER_PLANE // 2
                    for g in range(half):
                        emit_group(slab, pooled, g)
                    if pend is not None:
                        ER = emit_tail_a(pend)
                    for g in range(half, GRPS_PER_PLANE):
                        emit_group(slab, pooled, g)
                    if pend is not None:
                        emit_tail_b(pend, *ER)
                        if done_b is not None:
                            emit_b_final(*done_b)
                            done_b = None
                    pend = (pooled, exts[b], d * BLKS_PER_PLANE)
                    if d == D - 1:
                        done_b = (b, exts[b])
            # drain: last plane's tail + final b output
            ER = emit_tail_a(pend)
            emit_tail_b(pend, *ER)
            emit_b_final(*done_b)

    nc.compile()
    return nc


def _get_nc(passes=1):
    key = f"nc{passes}"
    if key not in _COMPILED:
        _COMPILED[key] = build_kernel(passes)
    return _COMPILED[key]


LAST_EXEC_NS = None


def build_in_maps(xs, wr, bias, subtract):
    # biasrep/subrep: [128, 1024] pattern (32 blocks x 32 ch), f32
    biasrep = np.tile(bias[None, None, :], (128, 32, 1)).reshape(
        128, 1024).astype(np.float32)
    subrep = np.tile(subtract[None, None, :], (128, 32, 1)).reshape(
        128, 1024).astype(np.float32)

    in_maps = []
    for c in range(N_CORES):
        in_maps.append({
            "xs": np.ascontiguousarray(xs[c * B_PER_CORE:(c + 1) * B_PER_CORE]),
            "wr": wr,
            "biasrep": biasrep,
            "subrep": subrep,
        })
    return in_maps


def kernel(x, weight, bias, subtract):
    from concourse.bass_utils import run_bass_kernel_spmd

    x = np.asarray(x, dtype=np.float32)
    weight = np.asarray(weight, dtype=np.float32)
    bias = np.asarray(bias, dtype=np.float32)
    subtract = np.asarray(subtract, dtype=np.float32)

    nc = _get_nc()

    xs = build_xstack(x)                      # [B, D, 128, PLANE]
    wr = build_wrhs(weight).astype(W_NP_DT)   # [128, 256]
    in_maps = build_in_maps(xs, wr, bias, subtract)

    res = run_bass_kernel_spmd(nc, in_maps, core_ids=list(range(N_CORES)))
    outs = [res.results[c]["y"].reshape(B_PER_CORE, D, H, W)
            for c in range(N_CORES)]
    return np.concatenate(outs, axis=0)
